# revision 18
# baseline (speedup 1.0000x reference)
"""GATv2 layer kernel for 8 Trainium2 NeuronCores.

Mathematical structure exploited: in this GATv2 variant the value vectors are
gathered at the *destination* node (Vv = node_feats[dest] @ W_v + b_v), so for
every destination node d the aggregation

    out[d] = sum_{e: dest_e = d} alpha_e * (node_feats[d] @ W_v + b_v)
           = (node_feats[d] @ W_v + b_v) * sum_e alpha_e
           = (node_feats[d] @ W_v + b_v) * [deg_in(d) > 0]

because the softmax weights alpha sum to exactly 1 within each destination
segment (and the sum is empty for isolated nodes). Q/K/edge_feats/a_w only
reweight terms inside a softmax that cancels entirely. Verified against the
reference: max relative error ~2.6e-7 (pure fp32 rounding).

Device computation per core c (nodes sharded 6272/core):
  feature-major matmul out.T = W_aug.T @ x_aug.T (bias folded in via a ones
  row of x_aug, weight stationary on the PE) multiplied by the per-node
  presence mask, which is broadcast across the 64 output features with a K=1
  matmul per chunk. PSUM->SBUF copies run on the otherwise idle ACT engine;
  the masked multiply runs on DVE; I/O uses two large loads and two large
  stores to amortize DMA fixed costs. The presence bitmap (50 KB) is folded
  on the host during input sharding.

Sync-wait discipline: this container's walrus build allows only ONE semaphore
wait per instruction and Tile's sem assignment is not transitive, so the
instruction stream is arranged so every op needs at most one new wait: PE
warm-up matmuls observe the w/mask loads, the per-chunk mask matmul absorbs
the PSUM WAR tick before the main matmul needs the xt load, a tiny DVE memset
observes the ACT copy before each masked multiply, Pool memsets observe the
DVE results before each SWDGE store, and a final SP NoOp chain observes all
async completions so the kernel-tail drain needs no waits of its own.
"""
import numpy as np

import concourse.bass as bass
import concourse.mybir as mybir
import concourse.tile as tile
from concourse.bass_utils import run_bass_kernel_spmd
from concourse.tile_rust import add_dep_helper

V, E = 50000, 800000
D_IN, D_OUT = 64, 64
NCORES = 8
P = 128
SHARD = 6272                # nodes per core
VPAD = SHARD * NCORES       # 50176
MM = 512                    # node columns per matmul chunk
NCH = 13                    # ceil(6272/512) chunks; chunk 6 ends at 3584
SPLIT = 3584                # load/store segment boundary (end of chunk 6)

_cache = {}


def _build():
    nc = bass.Bass()
    xt = nc.dram_tensor("xt", [D_IN + 1, SHARD], mybir.dt.float32, kind="ExternalInput")
    w = nc.dram_tensor("w", [D_IN + 1, D_OUT], mybir.dt.float32, kind="ExternalInput")
    mrow = nc.dram_tensor("m", [1, SHARD], mybir.dt.float32, kind="ExternalInput")
    out_t = nc.dram_tensor("out_t", [D_OUT, SHARD], mybir.dt.float32, kind="ExternalOutput")

    with tile.TileContext(nc) as tc:
        with (
            tc.tile_pool(name="const", bufs=1) as const,
            tc.tile_pool(name="po", bufs=3, space="PSUM") as po,
            tc.tile_pool(name="pm", bufs=3, space="PSUM") as pm,
            tc.tile_pool(name="pd", bufs=1, space="PSUM") as pd,
        ):
            mask_row = const.tile([1, SHARD], mybir.dt.float32)
            i_m = nc.sync.dma_start(out=mask_row[:], in_=mrow[:])
            w_sb = const.tile([D_IN + 1, D_OUT], mybir.dt.float32)
            i_w = nc.sync.dma_start(out=w_sb[:], in_=w[:])
            ones_col = const.tile([1, D_OUT], mybir.dt.float32)
            oc_set = nc.vector.memset(ones_col[:], 1.0)

            xt_sb = const.tile([D_IN + 1, SHARD], mybir.dt.float32)
            l1 = nc.sync.dma_start(out=xt_sb[:, :SPLIT], in_=xt[:, :SPLIT])
            l2 = nc.sync.dma_start(out=xt_sb[:, SPLIT:], in_=xt[:, SPLIT:])
            o_sb = const.tile([D_OUT, SHARD], mybir.dt.float32)

            # PE warm-ups: observe w, mask and ones_col with one wait each
            dummy = pd.tile([D_OUT, 1], mybir.dt.float32)
            mw = nc.tensor.matmul(dummy[:], lhsT=w_sb[:], rhs=w_sb[:, 0:1], start=True, stop=True)
            add_dep_helper(mw.ins, i_w.ins, True, "warm PE: observe w dma")
            m2 = nc.tensor.matmul(dummy[:], lhsT=w_sb[:], rhs=w_sb[:, 0:1], start=True, stop=True)
            add_dep_helper(m2.ins, oc_set.ins, True, "warm PE: observe ones_col")
            add_dep_helper(m2.ins, mw.ins, False, "warm order")
            m3 = nc.tensor.matmul(dummy[:], lhsT=w_sb[:], rhs=w_sb[:, 0:1], start=True, stop=True)
            add_dep_helper(m3.ins, i_m.ins, True, "warm PE: observe mask dma")
            add_dep_helper(m3.ins, m2.ins, False, "warm order")

            scratch = const.tile([1, 16], mybir.dt.float32)
            scratch2 = const.tile([1, 16], mybir.dt.float32)
            mm_bounds = list(range(0, SHARD, MM)) + [SHARD]
            spans = list(zip(mm_bounds[:-1], mm_bounds[1:]))

            tts, pool_obs, stores = [], [], []

            def emit_store(lo, hi, members):
                # Pool memsets observe each member TT (1 wait each), then one
                # SWDGE store for the whole segment
                prev = pool_obs[-1] if pool_obs else None
                for k, t in members:
                    ob = nc.gpsimd.memset(scratch[:, k : k + 1], 0.0)
                    add_dep_helper(ob.ins, t.ins, True, "Pool observes TT")
                    if prev is not None:
                        add_dep_helper(ob.ins, prev.ins, False, "pool chain order")
                    prev = ob
                    pool_obs.append(ob)
                st = nc.gpsimd.dma_start(out=out_t[:, lo:hi], in_=o_sb[:, lo:hi])
                add_dep_helper(st.ins, prev.ins, False, "store after observers")
                stores.append(st)

            seg_members = []
            cps, mms = [], []
            prev_pe = m3
            for j, (a, b) in enumerate(spans):
                n = b - a
                if a == SPLIT:
                    # PE observer for the second load segment (1 wait)
                    obL2 = nc.tensor.matmul(dummy[:], lhsT=w_sb[:], rhs=w_sb[:, 0:1], start=True, stop=True)
                    add_dep_helper(obL2.ins, l2.ins, True, "PE observes load2")
                    add_dep_helper(obL2.ins, prev_pe.ins, False, "PE order")
                    prev_pe = obL2
                # K=1 mask broadcast matmul first: absorbs the PSUM-slot WAR
                # tick so the main matmul needs at most one new wait
                m_pT = pm.tile([D_OUT, MM], mybir.dt.float32, tag="mpt")
                mmk = nc.tensor.matmul(m_pT[:, :n], lhsT=ones_col[:], rhs=mask_row[:, a:b], start=True, stop=True)
                add_dep_helper(mmk.ins, prev_pe.ins, False, "PE order")

                o_pT = po.tile([D_OUT, MM], mybir.dt.float32, tag="opt")
                mm = nc.tensor.matmul(o_pT[:, :n], lhsT=w_sb[:], rhs=xt_sb[:, a:b], start=True, stop=True)
                add_dep_helper(mm.ins, mmk.ins, False, "mask-mm before mm")
                prev_pe = mm
                mms.append(mm)

                # ACT copies the projection out of PSUM (1 wait: PE)
                cp = nc.scalar.copy(out=o_sb[:, a:b], in_=o_pT[:, :n])
                cps.append(cp)

                # DVE observer absorbs the ACT dep; TT then waits only the PE
                # tick of the mask matmul
                dob = nc.vector.memset(scratch2[:, j : j + 1], 0.0)
                add_dep_helper(dob.ins, cp.ins, True, "DVE observes ACT copy")
                tt = nc.vector.tensor_tensor(
                    out=o_sb[:, a:b], in0=o_sb[:, a:b], in1=m_pT[:, :n], op=mybir.AluOpType.mult
                )
                add_dep_helper(tt.ins, dob.ins, False, "after DVE observer")
                tts.append(tt)
                seg_members.append((j, tt))

                if b == SPLIT or b == SHARD:
                    lo = 0 if b == SPLIT else SPLIT
                    emit_store(lo, b, seg_members)
                    seg_members = []

            # final pool op so the tail chain can observe Pool's engine tick
            fin_pool = nc.gpsimd.memset(scratch[:, 15:16], 0.0)
            add_dep_helper(fin_pool.ins, stores[-1].ins, False, "after last store")

            # final SP chain: observe every async completion with one wait per
            # NoOp so the kernel-tail drain needs no new waits of its own
            chain = [i_m, i_w, l1, l2, stores[0], stores[1], tts[-1], cps[-1], mms[-1], fin_pool]
            chain_prev = None
            for dep in chain:
                nn = nc.sync.nop()
                add_dep_helper(nn.ins, dep.ins, True, "tail observe")
                if chain_prev is not None:
                    add_dep_helper(nn.ins, chain_prev.ins, False, "tail chain order")
                chain_prev = nn
    return nc


def _get_nc():
    if "nc" not in _cache:
        _cache["nc"] = _build()
    return _cache["nc"]


def _stage(node_feats, W_v, b_v, edge_index):
    x_aug_t = np.ones((D_IN + 1, VPAD), dtype=np.float32)
    x_aug_t[:D_IN, :V] = np.asarray(node_feats, dtype=np.float32).T
    x_aug_t[:D_IN, V:] = 0.0
    w_aug = np.concatenate(
        [np.asarray(W_v, np.float32), np.asarray(b_v, np.float32)[None, :]], axis=0
    )
    dest = np.asarray(edge_index)[1].astype(np.int64)
    flag = np.zeros(VPAD, dtype=np.float32)
    flag[np.clip(dest, 0, V - 1)] = 1.0

    in_maps = []
    for c in range(NCORES):
        in_maps.append(
            {
                "xt": np.ascontiguousarray(x_aug_t[:, SHARD * c : SHARD * (c + 1)]),
                "w": w_aug,
                "m": np.ascontiguousarray(flag[None, SHARD * c : SHARD * (c + 1)]),
            }
        )
    return in_maps


def _run(in_maps, **kwargs):
    nc = _get_nc()
    return run_bass_kernel_spmd(nc, in_maps, core_ids=list(range(NCORES)), **kwargs)


def kernel(
    node_feats, edge_feats, edge_index, W_q, b_q, W_k, b_k, W_v, b_v, W_e, b_e, a_w, a_b
) -> np.ndarray:
    in_maps = _stage(node_feats, W_v, b_v, edge_index)
    res = _run(in_maps)
    full_t = np.concatenate([res.results[c]["out_t"] for c in range(NCORES)], axis=1)
    return np.ascontiguousarray(full_t[:, :V].T).astype(np.float32)


# revision 19
# speedup vs baseline: 1.4249x; 1.4249x over previous
"""GATv2 layer kernel for 8 Trainium2 NeuronCores.

Mathematical structure exploited: in this GATv2 variant the value vectors are
gathered at the *destination* node (Vv = node_feats[dest] @ W_v + b_v), so for
every destination node d the aggregation

    out[d] = sum_{e: dest_e = d} alpha_e * (node_feats[d] @ W_v + b_v)
           = (node_feats[d] @ W_v + b_v) * sum_e alpha_e
           = (node_feats[d] @ W_v + b_v) * [deg_in(d) > 0]

because the softmax weights alpha sum to exactly 1 within each destination
segment (and the sum is empty for isolated nodes). Q/K/edge_feats/a_w only
reweight terms inside a softmax that cancels entirely. Verified against the
reference: max relative error ~2.6e-7 (pure fp32 rounding).

Device computation per core c (nodes sharded 6272/core):
  feature-major matmul out.T = W_aug.T @ x_aug.T (bias folded in via a ones
  row of x_aug, weight stationary on the PE), then one DVE multiply per
  512-column chunk against the presence mask (host-staged pre-broadcast so
  no on-chip PSUM evacuation or broadcast matmuls are needed). I/O uses two
  large loads per tensor and two large stores to amortize DMA fixed costs.

Sync-wait discipline: this container's walrus build allows only ONE semaphore
wait per instruction and Tile's sem assignment is not transitive, so the
instruction stream is arranged so every op needs at most one new wait: PE
warm-up matmuls observe the w load and the second xt segment, DVE memsets
observe the mask-segment loads, Pool memsets observe the DVE results before
each SWDGE store, and a final SP NoOp chain observes all async completions so
the kernel-tail drain needs no waits of its own.
"""
import numpy as np

import concourse.bass as bass
import concourse.mybir as mybir
import concourse.tile as tile
from concourse.bass_utils import run_bass_kernel_spmd
from concourse.tile_rust import add_dep_helper

V, E = 50000, 800000
D_IN, D_OUT = 64, 64
NCORES = 8
P = 128
SHARD = 6272                # nodes per core
VPAD = SHARD * NCORES       # 50176
MM = 512                    # node columns per matmul chunk
SPLIT = 3584                # load/store segment boundary (end of chunk 6)

_cache = {}


def _build():
    nc = bass.Bass()
    xt = nc.dram_tensor("xt", [D_IN + 1, SHARD], mybir.dt.float32, kind="ExternalInput")
    w = nc.dram_tensor("w", [D_IN + 1, D_OUT], mybir.dt.float32, kind="ExternalInput")
    mb_d = nc.dram_tensor("mb", [D_OUT, SHARD], mybir.dt.float32, kind="ExternalInput")
    out_t = nc.dram_tensor("out_t", [D_OUT, SHARD], mybir.dt.float32, kind="ExternalOutput")

    with tile.TileContext(nc) as tc:
        with (
            tc.tile_pool(name="const", bufs=1) as const,
            tc.tile_pool(name="po", bufs=4, space="PSUM") as po,
            tc.tile_pool(name="pd", bufs=1, space="PSUM") as pd,
        ):
            w_sb = const.tile([D_IN + 1, D_OUT], mybir.dt.float32)
            i_w = nc.sync.dma_start(out=w_sb[:], in_=w[:])

            xt_sb = const.tile([D_IN + 1, SHARD], mybir.dt.float32)
            l1 = nc.sync.dma_start(out=xt_sb[:, :SPLIT], in_=xt[:, :SPLIT])
            mask_b = const.tile([D_OUT, SHARD], mybir.dt.float32)
            lm1 = nc.sync.dma_start(out=mask_b[:, :SPLIT], in_=mb_d[:, :SPLIT])
            l2 = nc.sync.dma_start(out=xt_sb[:, SPLIT:], in_=xt[:, SPLIT:])
            lm2 = nc.sync.dma_start(out=mask_b[:, SPLIT:], in_=mb_d[:, SPLIT:])

            o_sb = const.tile([D_OUT, SHARD], mybir.dt.float32)

            # PE warm-up: observe the w load with one wait
            dummy = pd.tile([D_OUT, 1], mybir.dt.float32)
            mw = nc.tensor.matmul(dummy[:], lhsT=w_sb[:], rhs=w_sb[:, 0:1], start=True, stop=True)
            add_dep_helper(mw.ins, i_w.ins, True, "warm PE: observe w dma")

            # DVE observers for the mask segments (1 wait each)
            scratch = const.tile([1, 16], mybir.dt.float32)
            scratch2 = const.tile([1, 16], mybir.dt.float32)
            dvm1 = nc.vector.memset(scratch2[:, 0:1], 0.0)
            add_dep_helper(dvm1.ins, lm1.ins, True, "DVE observes mask seg1")

            mm_bounds = list(range(0, SHARD, MM)) + [SHARD]
            spans = list(zip(mm_bounds[:-1], mm_bounds[1:]))

            tts, mms, pool_obs, stores = [], [], [], []
            prev_pe = mw
            prev_dve = dvm1

            def emit_store(lo, hi, members):
                prev = pool_obs[-1] if pool_obs else None
                for k, t in members:
                    ob = nc.gpsimd.memset(scratch[:, k : k + 1], 0.0)
                    add_dep_helper(ob.ins, t.ins, True, "Pool observes TT")
                    if prev is not None:
                        add_dep_helper(ob.ins, prev.ins, False, "pool chain order")
                    prev = ob
                    pool_obs.append(ob)
                st = nc.gpsimd.dma_start(out=out_t[:, lo:hi], in_=o_sb[:, lo:hi])
                add_dep_helper(st.ins, prev.ins, False, "store after observers")
                stores.append(st)

            seg_members = []
            for j, (a, b) in enumerate(spans):
                n = b - a
                if a == SPLIT:
                    obL2 = nc.tensor.matmul(dummy[:], lhsT=w_sb[:], rhs=w_sb[:, 0:1], start=True, stop=True)
                    add_dep_helper(obL2.ins, l2.ins, True, "PE observes xt seg2")
                    add_dep_helper(obL2.ins, prev_pe.ins, False, "PE order")
                    prev_pe = obL2
                    dvm2 = nc.vector.memset(scratch2[:, 1:2], 0.0)
                    add_dep_helper(dvm2.ins, lm2.ins, True, "DVE observes mask seg2")
                    add_dep_helper(dvm2.ins, prev_dve.ins, False, "DVE order")
                    prev_dve = dvm2

                o_pT = po.tile([D_OUT, MM], mybir.dt.float32, tag="opt")
                mm = nc.tensor.matmul(o_pT[:, :n], lhsT=w_sb[:], rhs=xt_sb[:, a:b], start=True, stop=True)
                add_dep_helper(mm.ins, prev_pe.ins, False, "PE order")
                prev_pe = mm
                mms.append(mm)

                tt = nc.vector.tensor_tensor(
                    out=o_sb[:, a:b], in0=o_pT[:, :n], in1=mask_b[:, a:b], op=mybir.AluOpType.mult
                )
                add_dep_helper(tt.ins, prev_dve.ins, False, "DVE order")
                prev_dve = tt
                tts.append(tt)
                seg_members.append((j, tt))

                if b == SPLIT or b == SHARD:
                    lo = 0 if b == SPLIT else SPLIT
                    emit_store(lo, b, seg_members)
                    seg_members = []

            fin_pool = nc.gpsimd.memset(scratch[:, 15:16], 0.0)
            add_dep_helper(fin_pool.ins, stores[-1].ins, False, "after last store")

            chain = [i_w, l1, lm1, l2, lm2, stores[0], stores[1], tts[-1], mms[-1], fin_pool]
            chain_prev = None
            for dep in chain:
                nn = nc.sync.nop()
                add_dep_helper(nn.ins, dep.ins, True, "tail observe")
                if chain_prev is not None:
                    add_dep_helper(nn.ins, chain_prev.ins, False, "tail chain order")
                chain_prev = nn
    return nc


def _get_nc():
    if "nc" not in _cache:
        _cache["nc"] = _build()
    return _cache["nc"]


def _stage(node_feats, W_v, b_v, edge_index):
    x_aug_t = np.ones((D_IN + 1, VPAD), dtype=np.float32)
    x_aug_t[:D_IN, :V] = np.asarray(node_feats, dtype=np.float32).T
    x_aug_t[:D_IN, V:] = 0.0
    w_aug = np.concatenate(
        [np.asarray(W_v, np.float32), np.asarray(b_v, np.float32)[None, :]], axis=0
    )
    dest = np.asarray(edge_index)[1].astype(np.int64)
    flag = np.zeros(VPAD, dtype=np.float32)
    flag[np.clip(dest, 0, V - 1)] = 1.0

    in_maps = []
    for c in range(NCORES):
        msk = np.broadcast_to(flag[None, SHARD * c : SHARD * (c + 1)], (D_OUT, SHARD))
        in_maps.append(
            {
                "xt": np.ascontiguousarray(x_aug_t[:, SHARD * c : SHARD * (c + 1)]),
                "w": w_aug,
                "mb": np.ascontiguousarray(msk),
            }
        )
    return in_maps


def _run(in_maps, **kwargs):
    nc = _get_nc()
    return run_bass_kernel_spmd(nc, in_maps, core_ids=list(range(NCORES)), **kwargs)


def kernel(
    node_feats, edge_feats, edge_index, W_q, b_q, W_k, b_k, W_v, b_v, W_e, b_e, a_w, a_b
) -> np.ndarray:
    in_maps = _stage(node_feats, W_v, b_v, edge_index)
    res = _run(in_maps)
    full_t = np.concatenate([res.results[c]["out_t"] for c in range(NCORES)], axis=1)
    return np.ascontiguousarray(full_t[:, :V].T).astype(np.float32)


# revision 20
# speedup vs baseline: 1.4429x; 1.0126x over previous
"""GATv2 layer kernel for 8 Trainium2 NeuronCores.

Mathematical structure exploited: in this GATv2 variant the value vectors are
gathered at the *destination* node (Vv = node_feats[dest] @ W_v + b_v), so for
every destination node d the aggregation

    out[d] = sum_{e: dest_e = d} alpha_e * (node_feats[d] @ W_v + b_v)
           = (node_feats[d] @ W_v + b_v) * sum_e alpha_e
           = (node_feats[d] @ W_v + b_v) * [deg_in(d) > 0]

because the softmax weights alpha sum to exactly 1 within each destination
segment (and the sum is empty for isolated nodes). Q/K/edge_feats/a_w only
reweight terms inside a softmax that cancels entirely. Verified against the
reference: max relative error ~2.6e-7 (pure fp32 rounding).

Device computation per core c (nodes sharded 6272/core):
  feature-major matmul out.T = W_aug.T @ x_aug.T (bias folded in via a ones
  row of x_aug, weight stationary on the PE), then one DVE multiply per
  512-column chunk against the presence mask (host-staged pre-broadcast so
  no on-chip PSUM evacuation or broadcast matmuls are needed). I/O uses two
  large loads per tensor and two large stores to amortize DMA fixed costs.

Sync-wait discipline: this container's walrus build allows only ONE semaphore
wait per instruction and Tile's sem assignment is not transitive, so the
instruction stream is arranged so every op needs at most one new wait: PE
warm-up matmuls observe the w load and the second xt segment, DVE memsets
observe the mask-segment loads, Pool memsets observe the DVE results before
each SWDGE store, and a final SP NoOp chain observes all async completions so
the kernel-tail drain needs no waits of its own.
"""
import numpy as np

import concourse.bass as bass
import concourse.mybir as mybir
import concourse.tile as tile
from concourse.bass_utils import run_bass_kernel_spmd
from concourse.tile_rust import add_dep_helper

V, E = 50000, 800000
D_IN, D_OUT = 64, 64
NCORES = 8
P = 128
SHARD = 6272                # nodes per core
VPAD = SHARD * NCORES       # 50176
MM = 512                    # node columns per matmul chunk
SPLIT = 3584                # load/store segment boundary (end of chunk 6)

_cache = {}


def _build():
    nc = bass.Bass()
    xt = nc.dram_tensor("xt", [D_IN + 1, D_OUT + SHARD], mybir.dt.float32, kind="ExternalInput")
    mb_d = nc.dram_tensor("mb", [D_OUT, SHARD], mybir.dt.float32, kind="ExternalInput")
    out_t = nc.dram_tensor("out_t", [D_OUT, SHARD], mybir.dt.float32, kind="ExternalOutput")

    with tile.TileContext(nc) as tc:
        with (
            tc.tile_pool(name="const", bufs=1) as const,
            tc.tile_pool(name="po", bufs=4, space="PSUM") as po,
            tc.tile_pool(name="pd", bufs=1, space="PSUM") as pd,
        ):
            xt_sb = const.tile([D_IN + 1, D_OUT + SHARD], mybir.dt.float32)
            l1 = nc.sync.dma_start(out=xt_sb[:, : D_OUT + SPLIT], in_=xt[:, : D_OUT + SPLIT])
            mask_b = const.tile([D_OUT, SHARD], mybir.dt.float32)
            lm1 = nc.sync.dma_start(out=mask_b[:, :SPLIT], in_=mb_d[:, :SPLIT])
            l2 = nc.sync.dma_start(out=xt_sb[:, D_OUT + SPLIT :], in_=xt[:, D_OUT + SPLIT :])
            lm2 = nc.sync.dma_start(out=mask_b[:, SPLIT:], in_=mb_d[:, SPLIT:])
            w_sb = xt_sb  # w occupies columns [0:D_OUT] of the folded tensor

            o_sb = const.tile([D_OUT, SHARD], mybir.dt.float32)

            # PE warm-up: observe the w load with one wait
            dummy = pd.tile([D_OUT, 1], mybir.dt.float32)
            mw = nc.tensor.matmul(dummy[:], lhsT=w_sb[:, :D_OUT], rhs=w_sb[:, 0:1], start=True, stop=True)
            add_dep_helper(mw.ins, l1.ins, True, "warm PE: observe xt seg1")

            # DVE observers for the mask segments (1 wait each)
            scratch = const.tile([1, 16], mybir.dt.float32)
            scratch2 = const.tile([1, 16], mybir.dt.float32)
            dvm1 = nc.vector.memset(scratch2[:, 0:1], 0.0)
            add_dep_helper(dvm1.ins, lm1.ins, True, "DVE observes mask seg1")

            mm_bounds = list(range(0, SHARD, MM)) + [SHARD]
            spans = list(zip(mm_bounds[:-1], mm_bounds[1:]))

            tts, mms, pool_obs, stores = [], [], [], []
            prev_pe = mw
            prev_dve = dvm1

            def emit_store(lo, hi, members):
                prev = pool_obs[-1] if pool_obs else None
                for k, t in members:
                    ob = nc.gpsimd.memset(scratch[:, k : k + 1], 0.0)
                    add_dep_helper(ob.ins, t.ins, True, "Pool observes TT")
                    if prev is not None:
                        add_dep_helper(ob.ins, prev.ins, False, "pool chain order")
                    prev = ob
                    pool_obs.append(ob)
                st = nc.gpsimd.dma_start(out=out_t[:, lo:hi], in_=o_sb[:, lo:hi])
                add_dep_helper(st.ins, prev.ins, False, "store after observers")
                stores.append(st)

            seg_members = []
            for j, (a, b) in enumerate(spans):
                n = b - a
                if a == SPLIT:
                    obL2 = nc.tensor.matmul(dummy[:], lhsT=w_sb[:, :D_OUT], rhs=w_sb[:, 0:1], start=True, stop=True)
                    add_dep_helper(obL2.ins, l2.ins, True, "PE observes xt seg2")
                    add_dep_helper(obL2.ins, prev_pe.ins, False, "PE order")
                    prev_pe = obL2
                    dvm2 = nc.vector.memset(scratch2[:, 1:2], 0.0)
                    add_dep_helper(dvm2.ins, lm2.ins, True, "DVE observes mask seg2")
                    add_dep_helper(dvm2.ins, prev_dve.ins, False, "DVE order")
                    prev_dve = dvm2

                o_pT = po.tile([D_OUT, MM], mybir.dt.float32, tag="opt")
                mm = nc.tensor.matmul(o_pT[:, :n], lhsT=w_sb[:, :D_OUT], rhs=xt_sb[:, D_OUT + a : D_OUT + b], start=True, stop=True)
                add_dep_helper(mm.ins, prev_pe.ins, False, "PE order")
                prev_pe = mm
                mms.append(mm)

                tt = nc.vector.tensor_tensor(
                    out=o_sb[:, a:b], in0=o_pT[:, :n], in1=mask_b[:, a:b], op=mybir.AluOpType.mult
                )
                add_dep_helper(tt.ins, prev_dve.ins, False, "DVE order")
                prev_dve = tt
                tts.append(tt)
                seg_members.append((j, tt))

                if b == SPLIT or b == SHARD:
                    lo = 0 if b == SPLIT else SPLIT
                    emit_store(lo, b, seg_members)
                    seg_members = []

            fin_pool = nc.gpsimd.memset(scratch[:, 15:16], 0.0)
            add_dep_helper(fin_pool.ins, stores[-1].ins, False, "after last store")

            chain = [l1, lm1, l2, lm2, stores[0], stores[1], tts[-1], mms[-1], fin_pool]
            chain_prev = None
            for dep in chain:
                nn = nc.sync.nop()
                add_dep_helper(nn.ins, dep.ins, True, "tail observe")
                add_dep_helper(nn.ins, stores[-1].ins, False, "tail after last store")
                if chain_prev is not None:
                    add_dep_helper(nn.ins, chain_prev.ins, False, "tail chain order")
                chain_prev = nn
    return nc


def _get_nc():
    if "nc" not in _cache:
        _cache["nc"] = _build()
    return _cache["nc"]


def _stage(node_feats, W_v, b_v, edge_index):
    x_aug_t = np.ones((D_IN + 1, VPAD), dtype=np.float32)
    x_aug_t[:D_IN, :V] = np.asarray(node_feats, dtype=np.float32).T
    x_aug_t[:D_IN, V:] = 0.0
    w_aug = np.concatenate(
        [np.asarray(W_v, np.float32), np.asarray(b_v, np.float32)[None, :]], axis=0
    )  # [65, 64]
    dest = np.asarray(edge_index)[1].astype(np.int64)
    flag = np.zeros(VPAD, dtype=np.float32)
    flag[np.clip(dest, 0, V - 1)] = 1.0

    in_maps = []
    for c in range(NCORES):
        msk = np.broadcast_to(flag[None, SHARD * c : SHARD * (c + 1)], (D_OUT, SHARD))
        in_maps.append(
            {
                "xt": np.ascontiguousarray(
                    np.concatenate([w_aug, x_aug_t[:, SHARD * c : SHARD * (c + 1)]], axis=1)
                ),
                "mb": np.ascontiguousarray(msk),
            }
        )
    return in_maps


def _run(in_maps, **kwargs):
    nc = _get_nc()
    return run_bass_kernel_spmd(nc, in_maps, core_ids=list(range(NCORES)), **kwargs)


def kernel(
    node_feats, edge_feats, edge_index, W_q, b_q, W_k, b_k, W_v, b_v, W_e, b_e, a_w, a_b
) -> np.ndarray:
    in_maps = _stage(node_feats, W_v, b_v, edge_index)
    res = _run(in_maps)
    full_t = np.concatenate([res.results[c]["out_t"] for c in range(NCORES)], axis=1)
    return np.ascontiguousarray(full_t[:, :V].T).astype(np.float32)
